# revision 28
# baseline (speedup 1.0000x reference)
"""Multi-head attention TRN2 Bass kernel, sharded over 8 NeuronCores.

Sharding: core c -> (batch b = c//4, head-group g = c%4).  Each core computes
4 heads' worth of Q/K/V projections + attention for one batch element, plus
the partial output projection for its 256-column slice of the head-concat
dimension.  Host sums the 4 partials per batch and adds bf.

Key tricks:
  - All matmuls bf16 with fp32 PSUM accumulation.
  - Host pre-transposes x to [DIM, S] so contraction dims sit on partitions.
  - Attention is permutation-invariant over keys: the host sorts keys so
    unmasked tokens come first, and the kernel only processes the first
    NKT_A 128-token key chunks (fully-masked chunks contribute exactly 0).
  - Scores are computed transposed (S^T[kt, qt]); the two heads of a pair use
    disjoint PE row halves (K=64 at base partitions 0/64) and run concurrently.
  - pad_mask is folded into V and into an extra mask-column of V, so the AV
    matmul produces the masked numerator AND the softmax denominator, and exp
    needs no mask bias (raw scores are tiny; exp cannot overflow).
  - Fine-grained software pipelining: each head-pair's QK/exp loop also
    carries the previous pair's AV accumulation plus small projection /
    output-projection work units, keeping PE dense.
"""

import os
import numpy as np
import ml_dtypes

B, S, DIM, H, DH = 2, 2048, 1024, 16, 64
NCORES = 8
HPC = 4           # heads per core
CSL = HPC * DH    # 256: per-core slice of the head-concat dim
P = 128
KO = DIM // P     # 8 contraction chunks for projections
CC = CSL // P     # 2 col chunks (2 head-pairs)
NKT = S // P      # 16 key-token chunks (full)
QT = 512          # query tile (free dim)
NQT = S // QT     # 4 query tiles

BF16 = ml_dtypes.bfloat16

_CACHE = {}
LAST_RESULTS = None


def _build(nkt_a):
    import concourse.bass as bass
    import concourse.tile as tile
    from concourse import bacc, mybir
    from concourse.bass import ts

    f32 = mybir.dt.float32
    bf16 = mybir.dt.bfloat16

    KTILES = (nkt_a + 3) // 4          # 512-token K-projection tiles
    KTOK = KTILES * QT                 # padded key-token extent

    nc = bacc.Bacc("TRN2", target_bir_lowering=False, debug=False)

    xq = nc.dram_tensor("xq", [DIM, S], bf16, kind="ExternalInput").ap()
    xk = nc.dram_tensor("xk", [DIM, KTOK], bf16, kind="ExternalInput").ap()
    xv = nc.dram_tensor("xv", [DIM, KTOK], bf16, kind="ExternalInput").ap()
    wq = nc.dram_tensor("wq", [DIM, CSL], bf16, kind="ExternalInput").ap()
    wk = nc.dram_tensor("wk", [DIM, CSL], bf16, kind="ExternalInput").ap()
    wv = nc.dram_tensor("wv", [DIM, CSL], bf16, kind="ExternalInput").ap()
    wf = nc.dram_tensor("wf", [CSL, DIM], bf16, kind="ExternalInput").ap()
    bq = nc.dram_tensor("bq", [CSL], f32, kind="ExternalInput").ap()
    bk = nc.dram_tensor("bk", [CSL], f32, kind="ExternalInput").ap()
    bv = nc.dram_tensor("bv", [CSL], f32, kind="ExternalInput").ap()
    m01 = nc.dram_tensor("m01", [nkt_a * P], f32, kind="ExternalInput").ap()
    y = nc.dram_tensor("y", [S, DIM], f32, kind="ExternalOutput").ap()

    Exp = mybir.ActivationFunctionType.Exp
    MUL = mybir.AluOpType.mult

    with tile.TileContext(nc) as tc:
        with (
            tc.tile_pool(name="const", bufs=1) as const,
            tc.tile_pool(name="xql", bufs=4) as xql_pool,
            tc.tile_pool(name="xvl", bufs=4) as xvl_pool,
            tc.tile_pool(name="qkv", bufs=1) as qkv,
            tc.tile_pool(name="es", bufs=3) as es_pool,
            tc.tile_pool(name="ot", bufs=3) as ot_pool,
            tc.tile_pool(name="ysb", bufs=4) as ysb_pool,
            tc.tile_pool(name="rc", bufs=3) as rc_pool,
            tc.tile_pool(name="dscr", bufs=2, space="DRAM") as dram_pool,
            tc.tile_pool(name="stp", bufs=1, space="PSUM") as st_psum,
            tc.tile_pool(name="avp", bufs=2, space="PSUM") as av_psum,
            tc.tile_pool(name="mmp", bufs=2, space="PSUM") as mm_psum,
        ):
            # ---- constants (wk/bk first so the lead K-proj starts asap) ----
            wk_sb = const.tile([P, KO, CSL], bf16)
            wq_sb = const.tile([P, KO, CSL], bf16)
            wv_sb = const.tile([P, KO, CSL], bf16)
            wf_sb = const.tile([P, CC, DIM], bf16)
            bk_sb = const.tile([P, CC], f32)
            bq_sb = const.tile([P, CC], f32)
            bv_sb = const.tile([P, CSL], f32)
            m01_sb = const.tile([P, nkt_a], f32)
            nc.sync.dma_start(wk_sb, wk.rearrange("(ko p) e -> p ko e", p=P))
            nc.sync.dma_start(bk_sb, bk.rearrange("(cc p) -> p cc", p=P))

            xq_r = xq.rearrange("(ko p) s -> p ko s", p=P)
            xk_r = xk.rearrange("(ko p) s -> p ko s", p=P)
            xv_r = xv.rearrange("(ko p) s -> p ko s", p=P)
            y_r = y.rearrange("(t p) e -> t p e", p=P)

            qt_sb = qkv.tile([P, CC, S], bf16)
            kt_sb = qkv.tile([P, CC, KTOK], bf16)


            # V in AV-stationary form. Per head h, vaug[:, kc, h, :] is 128
            # wide: even h -> [V(64) | m01 | 0..], odd h -> [m01 | 0(63) | V(64)].
            # AV psum rows: even: O at 0..63, denom at 64;
            #               odd:  denom at 0, O at 64..127.
            vaug = qkv.tile([P, nkt_a, HPC, P], bf16)
            vaug_v = vaug.rearrange("p c (hp par) w -> p par c hp w", par=2)
            bv_v = bv_sb.rearrange("p (hp par d) -> p par hp d", par=2, d=DH)

            xql_cache = {}

            def emit_kproj_mini(c, cc):
                """One (128-token, 128-col) block of the K^T projection.
                Small blocks shorten the critical path to the first QK matmul
                and interleave finely into the first pair's kc loop."""
                key = ("xkm", c)
                xt = xql_cache.get(key)
                if xt is None:
                    xt = xvl_pool.tile([P, KO, P], bf16, tag="xvl",
                                       name=f"xkm{c}_{cc}")
                    nc.gpsimd.dma_start(xt, xk_r[:, :, ts(c, P)])
                    xql_cache[key] = xt
                    if len(xql_cache) > 3:
                        del xql_cache[next(iter(xql_cache))]
                ps = mm_psum.tile([P, P], f32, tag="mmp", name=f"km{c}_{cc}")
                for ko in range(KO):
                    nc.tensor.matmul(
                        ps, lhsT=wk_sb[:, ko, ts(cc, P)], rhs=xt[:, ko, :],
                        start=(ko == 0), stop=(ko == KO - 1),
                    )
                nc.vector.tensor_add(
                    out=kt_sb[:, cc, ts(c, P)], in0=ps,
                    in1=bk_sb[:, cc, None].to_broadcast((P, P)),
                )

            def emit_kq_proj(x_r, w_sb, b_sb, dst, t, cc):
                """One (512-token, 128-col) block of the K^T / Q^T projection.

                The x tile is shared between the two cc blocks of the same t
                via a small cache riding on the pool's buffer rotation."""
                key = (x_r.tensor.name, t)
                xt = xql_cache.get(key)
                if xt is None:
                    xt = xql_pool.tile([P, KO, QT], bf16, tag="xql",
                                       name=f"x{dst.tensor.name[:2]}_{t}_{cc}")
                    nc.gpsimd.dma_start(xt, x_r[:, :, ts(t, QT)])
                    xql_cache[key] = xt
                    if len(xql_cache) > 3:
                        del xql_cache[next(iter(xql_cache))]
                ps = mm_psum.tile([P, QT], f32, tag="mmp", name=f"pp{t}_{cc}")
                for ko in range(KO):
                    nc.tensor.matmul(
                        ps, lhsT=w_sb[:, ko, ts(cc, P)], rhs=xt[:, ko, :],
                        start=(ko == 0), stop=(ko == KO - 1),
                    )
                nc.vector.tensor_add(
                    out=dst[:, cc, ts(t, QT)], in0=ps,
                    in1=b_sb[:, cc, None].to_broadcast((P, QT)),
                )

            def emit_vproj_chunk(t):
                """One 128-token chunk of the V projection into vaug."""
                xt = xvl_pool.tile([P, KO, P], bf16, tag="xvl", name=f"xv_{t}")
                nc.gpsimd.dma_start(xt, xv_r[:, :, ts(t, P)])
                ps = mm_psum.tile([P, CSL], f32, tag="mmp", name=f"vp{t}")
                for ko in range(KO):
                    nc.tensor.matmul(
                        ps, lhsT=xt[:, ko, :], rhs=wv_sb[:, ko, :],
                        start=(ko == 0), stop=(ko == KO - 1),
                    )
                ps_v = ps.rearrange("p (hp par d) -> p par hp d", par=2, d=DH)
                for par, dlo in ((0, 0), (1, DH)):
                    dst = vaug_v[:, par, t, :, dlo:dlo + DH]
                    nc.vector.tensor_add(
                        out=dst, in0=ps_v[:, par, :, :], in1=bv_v[:, par, :, :],
                    )
                    nc.vector.tensor_tensor(
                        out=dst, in0=dst,
                        in1=m01_sb[:, t, None, None].to_broadcast((P, 2, DH)),
                        op=MUL,
                    )

            def emit_f_unit(t, tt, eh):
                """One [128 tok, 512 e] block of the output projection."""
                tok = t * (QT // P) + tt
                ps = mm_psum.tile([P, 512], f32, tag="mmp", name=f"fp{tok}_{eh}")
                for cc in range(CC):
                    nc.tensor.matmul(
                        ps, lhsT=ots[t][:, cc, ts(tt, P)],
                        rhs=wf_sb[:, cc, ts(eh, 512)],
                        start=(cc == 0), stop=(cc == CC - 1),
                    )
                ysb = ysb_pool.tile([P, 512], f32, tag="ysb", name=f"ys{tok}_{eh}")
                nc.vector.tensor_copy(out=ysb, in_=ps)
                nc.sync.dma_start(y_r[tok, :, ts(eh, 512)], ysb)

            class PairState:
                """QK/exp products of one head pair, awaiting AV drain."""

                def __init__(self, t, j):
                    self.t, self.j = t, j
                    self.es = es_pool.tile([P, nkt_a, 2, QT], bf16, tag="es",
                                           name=f"es{t}_{j}")
                    self.avs = [
                        av_psum.tile([P, QT], f32, tag="avp",
                                     name=f"avp{t}_{j}_{jj}")
                        for jj in range(2)
                    ]
                    self.av_kc = 0
                    self.stg = None

                def av_step(self):
                    kc = self.av_kc
                    for jj in range(2):
                        nc.tensor.matmul(
                            self.avs[jj],
                            lhsT=vaug[:, kc, 2 * self.j + jj, :],
                            rhs=self.es[:, kc, jj, :],
                            start=(kc == 0), stop=(kc == nkt_a - 1),
                        )
                    self.av_kc += 1

                def av_drain(self, upto):
                    while self.av_kc < upto:
                        self.av_step()

                def stage(self):
                    """Copy AV psums to SBUF (on ACT, which has slack) so the
                    PSUM slots free ~1.5us after the AV drain instead of after
                    the slow normalize DMA chain."""
                    t, j = self.t, self.j
                    self.stg = [
                        rc_pool.tile([P, QT], f32, tag="stg", bufs=6,
                                     name=f"sg{t}{j}{jj}")
                        for jj in range(2)
                    ]
                    nc.vector.tensor_copy(
                        out=self.stg[0][0:DH + 1, :], in_=self.avs[0][0:DH + 1, :])
                    nc.vector.tensor_copy(out=self.stg[1], in_=self.avs[1])

            def normalize_t(t, p0, p1):
                """Batched softmax normalization for q-tile t (both pairs).

                Denominator rows live at staged partitions 64 (even head) and
                0 (odd head).  One DVE reciprocal for all four rows, then a
                DRAM round-trip to partition-broadcast (only DRAM APs may have
                stride-0 partition dims)."""
                rall = rc_pool.tile([4, QT], f32, tag="rall", name=f"ra{t}")
                rr = rc_pool.tile([4, QT], f32, tag="rr", name=f"rr{t}")
                for i, (p, jj, row) in enumerate(
                        ((p0, 0, DH), (p0, 1, 0), (p1, 0, DH), (p1, 1, 0))):
                    nc.sync.dma_start(
                        rall[i:i + 1, :], p.stg[jj][row:row + 1, :])
                nc.vector.reciprocal(rr[0:4, :], rall[0:4, :])
                den_d = dram_pool.tile([4, QT], f32, tag="dend", name=f"dd{t}")
                nc.sync.dma_start(den_d, rr[0:4, :])
                for j, p in ((0, p0), (1, p1)):
                    rcb = rc_pool.tile([P, QT], f32, tag="rcb", name=f"rb{t}{j}")
                    nc.sync.dma_start(
                        rcb[0:DH, :],
                        den_d[2 * j, None, :].to_broadcast((DH, QT)))
                    nc.sync.dma_start(
                        rcb[DH:P, :],
                        den_d[2 * j + 1, None, :].to_broadcast((DH, QT)))
                    nc.vector.tensor_tensor(
                        out=ots[t][0:DH, j, :], in0=p.stg[0][0:DH, :],
                        in1=rcb[0:DH, :], op=MUL,
                    )
                    nc.vector.tensor_tensor(
                        out=ots[t][DH:P, j, :], in0=p.stg[1][DH:P, :],
                        in1=rcb[DH:P, :], op=MUL,
                    )

            def emit_pair(t, j, units, drain=None, self_av=False):
                """QK+exp loop for pair (t, j), interleaving `units` and the
                AV drain of a previous pair (and optionally its own)."""
                st = PairState(t, j)
                nu = len(units)
                ei = 0
                stp = None
                for kc in range(nkt_a):
                    half = kc % 2
                    if half == 0:
                        stp = st_psum.tile([P, 2, 2, QT], f32, tag="stp",
                                           name=f"st{t}_{j}_{kc}")
                    nc.tensor.matmul(
                        stp[:, half, 0, :],
                        lhsT=kt_sb[0:DH, j, ts(kc, P)],
                        rhs=qt_sb[0:DH, j, ts(t, QT)],
                        start=True, stop=True,
                    )
                    nc.tensor.matmul(
                        stp[:, half, 1, :],
                        lhsT=kt_sb[DH:P, j, ts(kc, P)],
                        rhs=qt_sb[DH:P, j, ts(t, QT)],
                        start=True, stop=True,
                    )
                    # one exp per completed pair of key chunks (N=2048
                    # amortizes the ~350-cycle ACT op overhead)
                    if half == 1:
                        nc.scalar.activation(
                            out=st.es[:, kc - 1:kc + 1, :, :], in_=stp,
                            func=Exp, scale=1.0 / DH,
                        )
                    elif kc == nkt_a - 1:
                        nc.scalar.activation(
                            out=st.es[:, kc, :, :], in_=stp[:, 0, :, :],
                            func=Exp, scale=1.0 / DH,
                        )
                    target = (kc + 1) * nu // nkt_a
                    while ei < target:
                        units[ei]()
                        ei += 1
                    if drain is not None:
                        drain.av_drain(kc + 1)
                if drain is not None:
                    drain.av_drain(nkt_a)
                    drain.stage()
                if self_av:
                    st.av_drain(nkt_a)
                    st.stage()
                return st

            # ---- lead-in: just enough K/Q projection for the first pair ----
            nc.sync.dma_start(wq_sb, wq.rearrange("(ko p) e -> p ko e", p=P))
            nc.sync.dma_start(bq_sb, bq.rearrange("(cc p) -> p cc", p=P))
            emit_kq_proj(xq_r, wq_sb, bq_sb, qt_sb, 0, 0)
            emit_kproj_mini(0, 0)
            nc.sync.dma_start(wv_sb, wv.rearrange("(ko p) e -> p ko e", p=P))
            nc.sync.dma_start(bv_sb, bv[None, :].to_broadcast((P, CSL)))
            nc.sync.dma_start(m01_sb, m01.rearrange("(c p) -> p c", p=P))
            nc.sync.dma_start(wf_sb, wf.rearrange("(cc p) e -> p cc e", p=P))
            nc.vector.memset(vaug, 0.0)
            nc.vector.tensor_copy(
                out=vaug_v[:, 0, :, :, DH],
                in_=m01_sb[:, :, None].to_broadcast((P, nkt_a, 2)),
            )
            nc.vector.tensor_copy(
                out=vaug_v[:, 1, :, :, 0],
                in_=m01_sb[:, :, None].to_broadcast((P, nkt_a, 2)),
            )

            ots = {
                t: ot_pool.tile([P, CC, QT], bf16, tag="ot", name=f"ot{t}")
                for t in range(NQT)
            }

            # remaining projection blocks as interleavable units
            k_units = [
                (lambda c=c, cc=cc: emit_kproj_mini(c, cc))
                for c in range(nkt_a) for cc in range(CC) if not (c == 0 and cc == 0)
            ]
            q0c1 = [lambda: emit_kq_proj(xq_r, wq_sb, bq_sb, qt_sb, 0, 1)]
            v_units = [
                (lambda tt=tt: emit_vproj_chunk(tt)) for tt in range(nkt_a)
            ]

            def qproj_units(t):
                return [
                    (lambda cc=cc, tn=t: emit_kq_proj(
                        xq_r, wq_sb, bq_sb, qt_sb, tn, cc))
                    for cc in range(CC)
                ]

            def f_units(t):
                return [
                    (lambda tt=tt, eh=eh, tp=t: emit_f_unit(tp, tt, eh))
                    for tt in range(QT // P) for eh in range(2)
                ]

            # Unit placement: ot(t-1) is complete only at the END of pair
            # (t, 0) (which drains pair (t-1, 1)), so f(t-1) units go in pair
            # (t, 1).  Qproj(t+1) must precede pair (t+1, 0): put it in (t, 0).
            prev = None
            pairs = {}
            for t in range(NQT):
                if t == 0:
                    u0 = k_units + q0c1 + qproj_units(1)
                    u1 = v_units
                else:
                    u0 = qproj_units(t + 1) if t < NQT - 1 else []
                    u1 = f_units(t - 1)
                p0 = emit_pair(t, 0, u0, drain=prev)
                if t >= 1:
                    normalize_t(t - 1, pairs[t - 1], prev)
                p1 = emit_pair(t, 1, u1, drain=p0,
                               self_av=(t == NQT - 1))
                pairs[t] = p0
                prev = p1
            # tail: normalize the last q-tile, then its output projection
            normalize_t(NQT - 1, pairs[NQT - 1], prev)
            for tt in range(QT // P):
                for eh in range(2):
                    emit_f_unit(NQT - 1, tt, eh)

    nc.compile()
    return nc


def _get_nc(nkt_a):
    if nkt_a not in _CACHE:
        _CACHE[nkt_a] = _build(nkt_a)
    return _CACHE[nkt_a]


def kernel(**inputs):
    global LAST_RESULTS
    query = np.asarray(inputs["query"], np.float32)
    key = np.asarray(inputs["key"], np.float32)
    value = np.asarray(inputs["value"], np.float32)
    pad_mask = np.asarray(inputs["pad_mask"])
    training = int(np.asarray(inputs["training_status"]))
    Wq = np.asarray(inputs["Wq"], np.float32)
    Wk = np.asarray(inputs["Wk"], np.float32)
    Wv = np.asarray(inputs["Wv"], np.float32)
    Wf = np.asarray(inputs["Wf"], np.float32)
    bq = np.asarray(inputs["bq"], np.float32)
    bk = np.asarray(inputs["bk"], np.float32)
    bv = np.asarray(inputs["bv"], np.float32)
    bf = np.asarray(inputs["bf"], np.float32)

    # Per-batch key permutation: unmasked keys first.  Attention is
    # permutation-invariant over keys, and fully-masked key chunks contribute
    # exactly zero (mask is folded into V and the denominator column), so the
    # kernel only needs ceil(max_unmasked / 128) key chunks.
    m01_full = {}
    perms = {}
    n_act = 1
    for b in range(B):
        if training:
            m = (pad_mask[b, 0, 0, :] != 0).astype(np.float32)
        else:
            m = np.ones(S, np.float32)
        perm = np.argsort(-m, kind="stable")
        m01_full[b] = m[perm]
        perms[b] = perm
        n_act = max(n_act, int(np.ceil(m.sum() / P)))
    nkt_a = min(NKT, max(2, n_act))
    ktok = ((nkt_a + 3) // 4) * QT

    nc = _get_nc(nkt_a)


    def prep_kv(x, b):
        xp = x[b][perms[b]]  # [S, DIM] permuted
        out = np.zeros((ktok, DIM), np.float32)
        out[: min(ktok, S)] = xp[:ktok]
        return np.ascontiguousarray(out.T).astype(BF16)

    xT = {}
    for b in range(B):
        xT[("q", b)] = np.ascontiguousarray(query[b].T).astype(BF16)
        xT[("k", b)] = prep_kv(key, b)
        xT[("v", b)] = prep_kv(value, b)
        m = np.zeros(nkt_a * P, np.float32)
        n = min(nkt_a * P, S)
        m[:n] = m01_full[b][:n]
        m01_full[b] = m

    in_maps = []
    for c in range(NCORES):
        b, g = divmod(c, HPC)
        cs = slice(g * CSL, (g + 1) * CSL)
        in_maps.append({
            "xq": xT[("q", b)],
            "xk": xT[("k", b)],
            "xv": xT[("v", b)],
            "wq": np.ascontiguousarray(Wq[:, cs]).astype(BF16),
            "wk": np.ascontiguousarray(Wk[:, cs]).astype(BF16),
            "wv": np.ascontiguousarray(Wv[:, cs]).astype(BF16),
            "wf": np.ascontiguousarray(Wf[cs, :]).astype(BF16),
            "bq": np.ascontiguousarray(bq[cs]),
            "bk": np.ascontiguousarray(bk[cs]),
            "bv": np.ascontiguousarray(bv[cs]),
            "m01": m01_full[b],
        })

    from concourse.bass_utils import run_bass_kernel_spmd

    res = run_bass_kernel_spmd(nc, in_maps, core_ids=list(range(NCORES)))
    LAST_RESULTS = res

    out = np.zeros((B, S, DIM), np.float32)
    for c in range(NCORES):
        b = c // HPC
        out[b] += res.results[c]["y"]
    out += bf[None, None, :]
    return out


# revision 29
# speedup vs baseline: 1.1306x; 1.1306x over previous
"""Multi-head attention TRN2 Bass kernel, sharded over 8 NeuronCores.

Sharding: core c -> (batch b = c//4, head-group g = c%4).  Each core computes
4 heads' worth of Q/K/V projections + attention for one batch element, plus
the partial output projection for its 256-column slice of the head-concat
dimension.  Host sums the 4 partials per batch and adds bf.

Key tricks:
  - All matmuls bf16 with fp32 PSUM accumulation.
  - Host pre-transposes x to [DIM, S] so contraction dims sit on partitions.
  - Attention is permutation-invariant over keys: the host sorts keys so
    unmasked tokens come first, and the kernel only processes the first
    NKT_A 128-token key chunks (fully-masked chunks contribute exactly 0).
  - Scores are computed transposed (S^T[kt, qt]); the two heads of a pair use
    disjoint PE row halves (K=64 at base partitions 0/64) and run concurrently.
  - pad_mask is folded into V and into an extra mask-column of V, so the AV
    matmul produces the masked numerator AND the softmax denominator, and exp
    needs no mask bias (raw scores are tiny; exp cannot overflow).
  - Fine-grained software pipelining: each head-pair's QK/exp loop also
    carries the previous pair's AV accumulation plus small projection /
    output-projection work units, keeping PE dense.
"""

import os
import numpy as np
import ml_dtypes

B, S, DIM, H, DH = 2, 2048, 1024, 16, 64
NCORES = 8
HPC = 4           # heads per core
CSL = HPC * DH    # 256: per-core slice of the head-concat dim
P = 128
KO = DIM // P     # 8 contraction chunks for projections
CC = CSL // P     # 2 col chunks (2 head-pairs)
NKT = S // P      # 16 key-token chunks (full)
QT = 512          # query tile (free dim)
NQT = S // QT     # 4 query tiles

BF16 = ml_dtypes.bfloat16

_CACHE = {}
LAST_RESULTS = None


def _build(nkt_a):
    import concourse.bass as bass
    import concourse.tile as tile
    from concourse import bacc, mybir
    from concourse.bass import ts

    f32 = mybir.dt.float32
    bf16 = mybir.dt.bfloat16

    KTILES = (nkt_a + 3) // 4          # 512-token K-projection tiles
    KTOK = KTILES * QT                 # padded key-token extent

    nc = bacc.Bacc("TRN2", target_bir_lowering=False, debug=False)

    xq = nc.dram_tensor("xq", [DIM, S], bf16, kind="ExternalInput").ap()
    xk = nc.dram_tensor("xk", [DIM, KTOK], bf16, kind="ExternalInput").ap()
    xv = nc.dram_tensor("xv", [DIM, KTOK], bf16, kind="ExternalInput").ap()
    wq = nc.dram_tensor("wq", [DIM, CSL], bf16, kind="ExternalInput").ap()
    wk = nc.dram_tensor("wk", [DIM, CSL], bf16, kind="ExternalInput").ap()
    wv = nc.dram_tensor("wv", [DIM, CSL], bf16, kind="ExternalInput").ap()
    wf = nc.dram_tensor("wf", [CSL, DIM], bf16, kind="ExternalInput").ap()
    bq = nc.dram_tensor("bq", [CSL], f32, kind="ExternalInput").ap()
    bk = nc.dram_tensor("bk", [CSL], f32, kind="ExternalInput").ap()
    bv = nc.dram_tensor("bv", [CSL], f32, kind="ExternalInput").ap()
    m01 = nc.dram_tensor("m01", [nkt_a * P], f32, kind="ExternalInput").ap()
    y = nc.dram_tensor("y", [S, DIM], f32, kind="ExternalOutput").ap()

    Exp = mybir.ActivationFunctionType.Exp
    MUL = mybir.AluOpType.mult

    with tile.TileContext(nc) as tc:
        with (
            tc.tile_pool(name="const", bufs=1) as const,
            tc.tile_pool(name="xql", bufs=4) as xql_pool,
            tc.tile_pool(name="xvl", bufs=4) as xvl_pool,
            tc.tile_pool(name="qkv", bufs=1) as qkv,
            tc.tile_pool(name="es", bufs=3) as es_pool,
            tc.tile_pool(name="ot", bufs=3) as ot_pool,
            tc.tile_pool(name="ysb", bufs=4) as ysb_pool,
            tc.tile_pool(name="rc", bufs=3) as rc_pool,
            tc.tile_pool(name="dscr", bufs=2, space="DRAM") as dram_pool,
            tc.tile_pool(name="stp", bufs=2, space="PSUM") as st_psum,
            tc.tile_pool(name="avp", bufs=2, space="PSUM") as av_psum,
            tc.tile_pool(name="mmp", bufs=2, space="PSUM") as mm_psum,
        ):
            # ---- constants (wk/bk first so the lead K-proj starts asap) ----
            wk_sb = const.tile([P, KO, CSL], bf16)
            wq_sb = const.tile([P, KO, CSL], bf16)
            wv_sb = const.tile([P, KO, CSL], bf16)
            wf_sb = const.tile([P, CC, DIM], bf16)
            bk_sb = const.tile([P, CC], f32)
            bq_sb = const.tile([P, CC], f32)
            bv_sb = const.tile([P, CSL], f32)
            m01_sb = const.tile([P, nkt_a], f32)
            nc.sync.dma_start(wk_sb, wk.rearrange("(ko p) e -> p ko e", p=P))
            nc.sync.dma_start(bk_sb, bk.rearrange("(cc p) -> p cc", p=P))

            xq_r = xq.rearrange("(ko p) s -> p ko s", p=P)
            xk_r = xk.rearrange("(ko p) s -> p ko s", p=P)
            xv_r = xv.rearrange("(ko p) s -> p ko s", p=P)
            y_r = y.rearrange("(t p) e -> t p e", p=P)

            qt_sb = qkv.tile([P, CC, S], bf16)
            kt_sb = qkv.tile([P, CC, KTOK], bf16)


            # V in AV-stationary form. Per head h, vaug[:, kc, h, :] is 128
            # wide: even h -> [V(64) | m01 | 0..], odd h -> [m01 | 0(63) | V(64)].
            # AV psum rows: even: O at 0..63, denom at 64;
            #               odd:  denom at 0, O at 64..127.
            vaug = qkv.tile([P, nkt_a, HPC, P], bf16)
            vaug_v = vaug.rearrange("p c (hp par) w -> p par c hp w", par=2)
            bv_v = bv_sb.rearrange("p (hp par d) -> p par hp d", par=2, d=DH)

            xql_cache = {}

            def emit_kproj_mini(c, cc):
                """One (128-token, 128-col) block of the K^T projection.
                Small blocks shorten the critical path to the first QK matmul
                and interleave finely into the first pair's kc loop."""
                key = ("xkm", c)
                xt = xql_cache.get(key)
                if xt is None:
                    xt = xvl_pool.tile([P, KO, P], bf16, tag="xvl",
                                       name=f"xkm{c}_{cc}")
                    nc.gpsimd.dma_start(xt, xk_r[:, :, ts(c, P)])
                    xql_cache[key] = xt
                    if len(xql_cache) > 3:
                        del xql_cache[next(iter(xql_cache))]
                ps = mm_psum.tile([P, P], f32, tag="mmp", name=f"km{c}_{cc}")
                for ko in range(KO):
                    nc.tensor.matmul(
                        ps, lhsT=wk_sb[:, ko, ts(cc, P)], rhs=xt[:, ko, :],
                        start=(ko == 0), stop=(ko == KO - 1),
                    )
                nc.vector.tensor_add(
                    out=kt_sb[:, cc, ts(c, P)], in0=ps,
                    in1=bk_sb[:, cc, None].to_broadcast((P, P)),
                )

            def emit_kq_proj(x_r, w_sb, b_sb, dst, t, cc):
                """One (512-token, 128-col) block of the K^T / Q^T projection.

                The x tile is shared between the two cc blocks of the same t
                via a small cache riding on the pool's buffer rotation."""
                key = (x_r.tensor.name, t)
                xt = xql_cache.get(key)
                if xt is None:
                    xt = xql_pool.tile([P, KO, QT], bf16, tag="xql",
                                       name=f"x{dst.tensor.name[:2]}_{t}_{cc}")
                    nc.gpsimd.dma_start(xt, x_r[:, :, ts(t, QT)])
                    xql_cache[key] = xt
                    if len(xql_cache) > 3:
                        del xql_cache[next(iter(xql_cache))]
                ps = mm_psum.tile([P, QT], f32, tag="mmp", name=f"pp{t}_{cc}")
                for ko in range(KO):
                    nc.tensor.matmul(
                        ps, lhsT=w_sb[:, ko, ts(cc, P)], rhs=xt[:, ko, :],
                        start=(ko == 0), stop=(ko == KO - 1),
                    )
                nc.vector.tensor_add(
                    out=dst[:, cc, ts(t, QT)], in0=ps,
                    in1=b_sb[:, cc, None].to_broadcast((P, QT)),
                )

            def emit_vproj_chunk(t):
                """One 128-token chunk of the V projection into vaug."""
                xt = xvl_pool.tile([P, KO, P], bf16, tag="xvl", name=f"xv_{t}")
                nc.gpsimd.dma_start(xt, xv_r[:, :, ts(t, P)])
                ps = mm_psum.tile([P, CSL], f32, tag="mmp", name=f"vp{t}")
                for ko in range(KO):
                    nc.tensor.matmul(
                        ps, lhsT=xt[:, ko, :], rhs=wv_sb[:, ko, :],
                        start=(ko == 0), stop=(ko == KO - 1),
                    )
                ps_v = ps.rearrange("p (hp par d) -> p par hp d", par=2, d=DH)
                for par, dlo in ((0, 0), (1, DH)):
                    dst = vaug_v[:, par, t, :, dlo:dlo + DH]
                    nc.vector.tensor_add(
                        out=dst, in0=ps_v[:, par, :, :], in1=bv_v[:, par, :, :],
                    )
                    nc.vector.tensor_tensor(
                        out=dst, in0=dst,
                        in1=m01_sb[:, t, None, None].to_broadcast((P, 2, DH)),
                        op=MUL,
                    )

            def emit_f_unit(t, tt, eh):
                """One [128 tok, 512 e] block of the output projection."""
                tok = t * (QT // P) + tt
                ps = mm_psum.tile([P, 512], f32, tag="mmp", name=f"fp{tok}_{eh}")
                for cc in range(CC):
                    nc.tensor.matmul(
                        ps, lhsT=ots[t][:, cc, ts(tt, P)],
                        rhs=wf_sb[:, cc, ts(eh, 512)],
                        start=(cc == 0), stop=(cc == CC - 1),
                    )
                ysb = ysb_pool.tile([P, 512], f32, tag="ysb", name=f"ys{tok}_{eh}")
                nc.vector.tensor_copy(out=ysb, in_=ps)
                nc.sync.dma_start(y_r[tok, :, ts(eh, 512)], ysb)

            class PairState:
                """QK/exp products of one head pair, awaiting AV drain."""

                def __init__(self, t, j):
                    self.t, self.j = t, j
                    self.es = es_pool.tile([P, nkt_a, 2, QT], bf16, tag="es",
                                           name=f"es{t}_{j}")
                    self.avs = [
                        av_psum.tile([P, QT], f32, tag="avp",
                                     name=f"avp{t}_{j}_{jj}")
                        for jj in range(2)
                    ]
                    self.av_kc = 0
                    self.stg = None

                def av_step(self):
                    kc = self.av_kc
                    for jj in range(2):
                        nc.tensor.matmul(
                            self.avs[jj],
                            lhsT=vaug[:, kc, 2 * self.j + jj, :],
                            rhs=self.es[:, kc, jj, :],
                            start=(kc == 0), stop=(kc == nkt_a - 1),
                        )
                    self.av_kc += 1

                def av_drain(self, upto):
                    while self.av_kc < upto:
                        self.av_step()

                def stage(self):
                    """Copy AV psums to SBUF (on ACT, which has slack) so the
                    PSUM slots free ~1.5us after the AV drain instead of after
                    the slow normalize DMA chain."""
                    t, j = self.t, self.j
                    self.stg = [
                        rc_pool.tile([P, QT], f32, tag="stg", bufs=6,
                                     name=f"sg{t}{j}{jj}")
                        for jj in range(2)
                    ]
                    nc.vector.tensor_copy(
                        out=self.stg[0][0:DH + 1, :], in_=self.avs[0][0:DH + 1, :])
                    nc.vector.tensor_copy(out=self.stg[1], in_=self.avs[1])

            def normalize_t(t, p0, p1):
                """Batched softmax normalization for q-tile t (both pairs).

                Denominator rows live at staged partitions 64 (even head) and
                0 (odd head).  One DVE reciprocal for all four rows, then a
                DRAM round-trip to partition-broadcast (only DRAM APs may have
                stride-0 partition dims)."""
                rall = rc_pool.tile([4, QT], f32, tag="rall", name=f"ra{t}")
                rr = rc_pool.tile([4, QT], f32, tag="rr", name=f"rr{t}")
                for i, (p, jj, row) in enumerate(
                        ((p0, 0, DH), (p0, 1, 0), (p1, 0, DH), (p1, 1, 0))):
                    nc.sync.dma_start(
                        rall[i:i + 1, :], p.stg[jj][row:row + 1, :])
                nc.vector.reciprocal(rr[0:4, :], rall[0:4, :])
                den_d = dram_pool.tile([4, QT], f32, tag="dend", name=f"dd{t}")
                nc.sync.dma_start(den_d, rr[0:4, :])
                for j, p in ((0, p0), (1, p1)):
                    rcb = rc_pool.tile([P, QT], f32, tag="rcb", name=f"rb{t}{j}")
                    nc.sync.dma_start(
                        rcb[0:DH, :],
                        den_d[2 * j, None, :].to_broadcast((DH, QT)))
                    nc.sync.dma_start(
                        rcb[DH:P, :],
                        den_d[2 * j + 1, None, :].to_broadcast((DH, QT)))
                    nc.vector.tensor_tensor(
                        out=ots[t][0:DH, j, :], in0=p.stg[0][0:DH, :],
                        in1=rcb[0:DH, :], op=MUL,
                    )
                    nc.vector.tensor_tensor(
                        out=ots[t][DH:P, j, :], in0=p.stg[1][DH:P, :],
                        in1=rcb[DH:P, :], op=MUL,
                    )

            def emit_pair(t, j, units, drain=None, self_av=False):
                """QK+exp loop for pair (t, j), interleaving `units` and the
                AV drain of a previous pair (and optionally its own)."""
                st = PairState(t, j)
                nu = len(units)
                ei = 0
                for kc in range(nkt_a):
                    stp = st_psum.tile([P, 2, QT], f32, tag="stp",
                                       name=f"st{t}_{j}_{kc}")
                    nc.tensor.matmul(
                        stp[:, 0, :],
                        lhsT=kt_sb[0:DH, j, ts(kc, P)],
                        rhs=qt_sb[0:DH, j, ts(t, QT)],
                        start=True, stop=True,
                    )
                    nc.tensor.matmul(
                        stp[:, 1, :],
                        lhsT=kt_sb[DH:P, j, ts(kc, P)],
                        rhs=qt_sb[DH:P, j, ts(t, QT)],
                        start=True, stop=True,
                    )
                    nc.scalar.activation(
                        out=st.es[:, kc, :, :], in_=stp[:, :, :],
                        func=Exp, scale=1.0 / DH,
                    )
                    target = (kc + 1) * nu // nkt_a
                    while ei < target:
                        units[ei]()
                        ei += 1
                    if drain is not None:
                        drain.av_drain(kc + 1)
                if drain is not None:
                    drain.av_drain(nkt_a)
                    drain.stage()
                if self_av:
                    st.av_drain(nkt_a)
                    st.stage()
                return st

            # ---- lead-in: just enough K/Q projection for the first pair ----
            nc.sync.dma_start(wq_sb, wq.rearrange("(ko p) e -> p ko e", p=P))
            nc.sync.dma_start(bq_sb, bq.rearrange("(cc p) -> p cc", p=P))
            emit_kq_proj(xq_r, wq_sb, bq_sb, qt_sb, 0, 0)
            emit_kproj_mini(0, 0)
            nc.sync.dma_start(wv_sb, wv.rearrange("(ko p) e -> p ko e", p=P))
            nc.sync.dma_start(bv_sb, bv[None, :].to_broadcast((P, CSL)))
            nc.sync.dma_start(m01_sb, m01.rearrange("(c p) -> p c", p=P))
            nc.sync.dma_start(wf_sb, wf.rearrange("(cc p) e -> p cc e", p=P))
            nc.vector.memset(vaug, 0.0)
            nc.vector.tensor_copy(
                out=vaug_v[:, 0, :, :, DH],
                in_=m01_sb[:, :, None].to_broadcast((P, nkt_a, 2)),
            )
            nc.vector.tensor_copy(
                out=vaug_v[:, 1, :, :, 0],
                in_=m01_sb[:, :, None].to_broadcast((P, nkt_a, 2)),
            )

            ots = {
                t: ot_pool.tile([P, CC, QT], bf16, tag="ot", name=f"ot{t}")
                for t in range(NQT)
            }

            # remaining projection blocks as interleavable units
            k_units = [
                (lambda c=c, cc=cc: emit_kproj_mini(c, cc))
                for c in range(nkt_a) for cc in range(CC) if not (c == 0 and cc == 0)
            ]
            q0c1 = [lambda: emit_kq_proj(xq_r, wq_sb, bq_sb, qt_sb, 0, 1)]
            v_units = [
                (lambda tt=tt: emit_vproj_chunk(tt)) for tt in range(nkt_a)
            ]

            def qproj_units(t):
                return [
                    (lambda cc=cc, tn=t: emit_kq_proj(
                        xq_r, wq_sb, bq_sb, qt_sb, tn, cc))
                    for cc in range(CC)
                ]

            def f_units(t):
                return [
                    (lambda tt=tt, eh=eh, tp=t: emit_f_unit(tp, tt, eh))
                    for tt in range(QT // P) for eh in range(2)
                ]

            # Unit placement: ot(t-1) is complete only at the END of pair
            # (t, 0) (which drains pair (t-1, 1)), so f(t-1) units go in pair
            # (t, 1).  Qproj(t+1) must precede pair (t+1, 0): put it in (t, 0).
            prev = None
            pairs = {}
            for t in range(NQT):
                if t == 0:
                    u0 = k_units + q0c1 + qproj_units(1)
                    u1 = v_units
                else:
                    u0 = qproj_units(t + 1) if t < NQT - 1 else []
                    u1 = f_units(t - 1)
                p0 = emit_pair(t, 0, u0, drain=prev)
                if t >= 1:
                    normalize_t(t - 1, pairs[t - 1], prev)
                p1 = emit_pair(t, 1, u1, drain=p0,
                               self_av=(t == NQT - 1))
                pairs[t] = p0
                prev = p1
            # tail: normalize the last q-tile, then its output projection
            normalize_t(NQT - 1, pairs[NQT - 1], prev)
            for tt in range(QT // P):
                for eh in range(2):
                    emit_f_unit(NQT - 1, tt, eh)

    nc.compile()
    return nc


def _get_nc(nkt_a):
    if nkt_a not in _CACHE:
        _CACHE[nkt_a] = _build(nkt_a)
    return _CACHE[nkt_a]


def kernel(**inputs):
    global LAST_RESULTS
    query = np.asarray(inputs["query"], np.float32)
    key = np.asarray(inputs["key"], np.float32)
    value = np.asarray(inputs["value"], np.float32)
    pad_mask = np.asarray(inputs["pad_mask"])
    training = int(np.asarray(inputs["training_status"]))
    Wq = np.asarray(inputs["Wq"], np.float32)
    Wk = np.asarray(inputs["Wk"], np.float32)
    Wv = np.asarray(inputs["Wv"], np.float32)
    Wf = np.asarray(inputs["Wf"], np.float32)
    bq = np.asarray(inputs["bq"], np.float32)
    bk = np.asarray(inputs["bk"], np.float32)
    bv = np.asarray(inputs["bv"], np.float32)
    bf = np.asarray(inputs["bf"], np.float32)

    # Per-batch key permutation: unmasked keys first.  Attention is
    # permutation-invariant over keys, and fully-masked key chunks contribute
    # exactly zero (mask is folded into V and the denominator column), so the
    # kernel only needs ceil(max_unmasked / 128) key chunks.
    m01_full = {}
    perms = {}
    n_act = 1
    for b in range(B):
        if training:
            m = (pad_mask[b, 0, 0, :] != 0).astype(np.float32)
        else:
            m = np.ones(S, np.float32)
        perm = np.argsort(-m, kind="stable")
        m01_full[b] = m[perm]
        perms[b] = perm
        n_act = max(n_act, int(np.ceil(m.sum() / P)))
    nkt_a = min(NKT, max(2, n_act))
    ktok = ((nkt_a + 3) // 4) * QT

    nc = _get_nc(nkt_a)


    def prep_kv(x, b):
        xp = x[b][perms[b]]  # [S, DIM] permuted
        out = np.zeros((ktok, DIM), np.float32)
        out[: min(ktok, S)] = xp[:ktok]
        return np.ascontiguousarray(out.T).astype(BF16)

    xT = {}
    for b in range(B):
        xT[("q", b)] = np.ascontiguousarray(query[b].T).astype(BF16)
        xT[("k", b)] = prep_kv(key, b)
        xT[("v", b)] = prep_kv(value, b)
        m = np.zeros(nkt_a * P, np.float32)
        n = min(nkt_a * P, S)
        m[:n] = m01_full[b][:n]
        m01_full[b] = m

    in_maps = []
    for c in range(NCORES):
        b, g = divmod(c, HPC)
        cs = slice(g * CSL, (g + 1) * CSL)
        in_maps.append({
            "xq": xT[("q", b)],
            "xk": xT[("k", b)],
            "xv": xT[("v", b)],
            "wq": np.ascontiguousarray(Wq[:, cs]).astype(BF16),
            "wk": np.ascontiguousarray(Wk[:, cs]).astype(BF16),
            "wv": np.ascontiguousarray(Wv[:, cs]).astype(BF16),
            "wf": np.ascontiguousarray(Wf[cs, :]).astype(BF16),
            "bq": np.ascontiguousarray(bq[cs]),
            "bk": np.ascontiguousarray(bk[cs]),
            "bv": np.ascontiguousarray(bv[cs]),
            "m01": m01_full[b],
        })

    from concourse.bass_utils import run_bass_kernel_spmd

    res = run_bass_kernel_spmd(nc, in_maps, core_ids=list(range(NCORES)))
    LAST_RESULTS = res

    out = np.zeros((B, S, DIM), np.float32)
    for c in range(NCORES):
        b = c // HPC
        out[b] += res.results[c]["y"]
    out += bf[None, None, :]
    return out
